# revision 18
# baseline (speedup 1.0000x reference)
"""Trainium2 Bass kernel for nn_NonSOCSymmetricContraction.

Math (reference):
  wy_o = einsum('ekqa,be->bkqa', w_o, y)             o in {1,2,3}
  t1[b,a] = sum_p coeff1[p] * x[b,a,i1,l1] * wy1[b,k1,q1,a]
  t2[b,a] = sum_p coeff2[p] * x[b,a,i2,l2] * x[b,a,j2,m2] * wy2[b,k2,q2,a]
  t3[b,a] = sum_p coeff3[p] * x[b,a,i3,l3] * x[b,a,j3,m3] * x[b,a,f3,g3] * wy3[b,k3,q3,a]
  out = t1 + t2 + t3                                  [B, A]

Device algorithm (per core, data-parallel over B; BL = B/8 = 512).
All matmul operands are bf16 (1 cyc/col on PE; fp32 is 4 cyc/col).
x is stored with even a's on partitions 0-63 and odd a's on 64-127, so an
(even, odd) a-pair shares one BL-column block and its matmuls run
concurrently on disjoint PE row/col groups (tile_position pairing).

Per a-pair (ae even rows, ao odd rows; j = a%4 selects the 32-row block
of the group accumulator c4 and the PE column strip):
  path1: fully-folded K=64 matmul (a1w) on xa -> c4.
  path2 (square trick, no DVE product):
      s   = 2hot-gather(x)    (one K=64 matmul; sel2 column has 1s at
                               c2a(p), c2b(p), or a single 2)
      s2  = ACT Square(s) PSUM->SBUF
      c4 += (wg2/2) @ s2      (K=128 contract)
      c4 -= fold(wg2/2) @ xsq (K=64 folded matmul; xsq = x*x via GpSimd)
      since x*y = ((x+y)^2 - x^2 - y^2)/2.
  path3 (2 chunks of 128 paths): 3 gathers ga,gb,gc (K=64 matmuls),
      ACT copy gb->SBUF, t = ga*gb_sb (DVE), x3 = gc*t (DVE),
      c4 += wg3 @ x3 (K=128 contract).
DVE/ACT ops are FD=1024 batched over the a-pair (gather pairs land in
adjacent PSUM banks). Group finalize: v = c4*y4 (DVE), block-sum matmul
with 0/1 sig collapses e, ACT copy, DMA out.
"""

import sys

import numpy as np

if "/opt/trn_rl_repo" not in sys.path:
    sys.path.insert(0, "/opt/trn_rl_repo")

B, A, L, M, E = 4096, 64, 16, 4, 10
NCORES = 8
BL = B // NCORES  # 512
P1, P2, P3 = 32, 128, 256
AG = 4  # a-values packed per PSUM accumulator
NG = A // AG  # 16 a-groups

_CACHE: dict = {}


def _build_module():
    """Build and compile the (input-independent) Bass module once."""
    import concourse.bacc as bacc
    import concourse.mybir as mybir
    from concourse import tile

    f32 = mybir.dt.float32
    bf16 = mybir.dt.bfloat16

    nc = bacc.Bacc(
        "TRN2",
        target_bir_lowering=False,
        debug=False,
        enable_asserts=False,
        num_devices=NCORES,
    )

    XW = (A // 2) * BL  # 16384
    xt_d = nc.dram_tensor("xt", [128, XW], bf16, kind="ExternalInput")
    y4_d = nc.dram_tensor("y4", [128, BL], bf16, kind="ExternalInput")
    sel2_d = nc.dram_tensor("sel2", [128, 128], bf16, kind="ExternalInput")
    sel3_d = nc.dram_tensor("sel3", [128, 768], bf16, kind="ExternalInput")
    a1w_d = nc.dram_tensor("a1w", [128, A * 32], bf16, kind="ExternalInput")
    sqw_d = nc.dram_tensor("sqw", [128, A * 32], bf16, kind="ExternalInput")
    wg2_d = nc.dram_tensor("wg2", [128, A * 32], bf16, kind="ExternalInput")
    wg3_d = nc.dram_tensor("wg3", [128, A * 64], bf16, kind="ExternalInput")
    sig_d = nc.dram_tensor("sig", [128, 4], bf16, kind="ExternalInput")
    out_d = nc.dram_tensor("out", [AG, NG * BL], f32, kind="ExternalOutput")

    XCH = 16  # DMA chunks for xt so compute can start early

    with tile.TileContext(nc) as tc:
        with (
            tc.tile_pool(name="const", bufs=1) as const,
            tc.tile_pool(name="work", bufs=6) as work,
            tc.tile_pool(name="psum_g", bufs=3, space="PSUM") as psum_g,
            tc.tile_pool(name="psum_c", bufs=1, space="PSUM") as psum_c,
            tc.tile_pool(name="psum_o", bufs=1, space="PSUM") as psum_o,
        ):
            # small consts first so pair-0 compute starts as soon as
            # the first xt chunk lands (xt is the bulk of input DMA)
            sel2 = const.tile([128, 128], bf16)
            nc.sync.dma_start(out=sel2[:], in_=sel2_d[:])
            sel3 = const.tile([128, 768], bf16)
            nc.sync.dma_start(out=sel3[:], in_=sel3_d[:])
            y4 = const.tile([128, BL], bf16)
            nc.sync.dma_start(out=y4[:], in_=y4_d[:])
            a1w = const.tile([128, A * 32], bf16)
            nc.sync.dma_start(out=a1w[:], in_=a1w_d[:])
            sqw = const.tile([128, A * 32], bf16)
            nc.sync.dma_start(out=sqw[:], in_=sqw_d[:])
            wg2 = const.tile([128, A * 32], bf16)
            nc.sync.dma_start(out=wg2[:], in_=wg2_d[:])
            wg3 = const.tile([128, A * 64], bf16)
            nc.sync.dma_start(out=wg3[:], in_=wg3_d[:])
            sig = const.tile([128, 4], bf16)
            nc.sync.dma_start(out=sig[:], in_=sig_d[:])
            xt = const.tile([128, XW], bf16)
            ach = XW // XCH
            for i in range(XCH):
                nc.sync.dma_start(
                    out=xt[:, i * ach : (i + 1) * ach],
                    in_=xt_d[:, i * ach : (i + 1) * ach],
                )

            # xsq = xt * xt on the otherwise-idle GpSimd engine
            xsq = const.tile([128, XW], bf16)
            SQCH = 16
            sch = XW // SQCH
            for i in range(SQCH):
                nc.gpsimd.tensor_mul(
                    xsq[:, i * sch : (i + 1) * sch],
                    xt[:, i * sch : (i + 1) * sch],
                    xt[:, i * sch : (i + 1) * sch],
                )

            # Software-pipelined emission. Engines execute their queues
            # IN ORDER, so program order defines the per-engine schedule.
            # Each "unit" is (gathers -> ACT/DVE chain -> contract MMs).
            # Unit k+1's gathers are emitted BEFORE unit k's contracts so
            # the PE never sits behind a semaphore wait for the DVE chain;
            # psum_g's per-tile buffer reuse provides incremental overlap.
            c4_of_group: dict = {}

            def group_c4(g):
                if g not in c4_of_group:
                    c4_of_group[g] = psum_c.tile([128, BL], f32, name="c4")
                return c4_of_group[g]

            def unit_p2(ap):
                ae, ao = 2 * ap, 2 * ap + 1
                je, jo = ae % AG, ao % AG
                xa_e = xt[0:64, ap * BL : (ap + 1) * BL]
                xa_o = xt[64:128, ap * BL : (ap + 1) * BL]

                def gathers():
                    s_pair = psum_g.tile([128, 2 * BL], f32, tag="gath")
                    nc.tensor.matmul(
                        s_pair[:, 0:BL], sel2[0:64, :], xa_e,
                        start=True, stop=True, tile_position=(0, 0),
                    )
                    nc.tensor.matmul(
                        s_pair[:, BL : 2 * BL], sel2[64:128, :], xa_o,
                        start=True, stop=True, tile_position=(64, 0),
                    )
                    return s_pair

                def chain(s_pair):
                    s2 = work.tile([128, 2 * BL], bf16, tag="s2")
                    nc.scalar.square(s2[:], s_pair[:])
                    return s2

                def contracts(s2):
                    c4 = group_c4(ae // AG)
                    cs_e = c4[32 * je : 32 * je + 32, :]
                    cs_o = c4[32 * jo : 32 * jo + 32, :]
                    nc.tensor.matmul(
                        c4[32 * je : 32 * je + 64, :],
                        a1w[:, ap * 64 : (ap + 1) * 64],
                        xt[:, ap * BL : (ap + 1) * BL],
                        start=False, stop=False, tile_position=(0, 32 * je),
                    )
                    nc.tensor.matmul(
                        cs_e, wg2[:, ae * 32 : (ae + 1) * 32], s2[:, 0:BL],
                        start=False, stop=False, tile_position=(0, 32 * je),
                    )
                    nc.tensor.matmul(
                        cs_o, wg2[:, ao * 32 : (ao + 1) * 32],
                        s2[:, BL : 2 * BL],
                        start=False, stop=False, tile_position=(0, 32 * jo),
                    )
                    nc.tensor.matmul(
                        c4[32 * je : 32 * je + 64, :],
                        sqw[:, ap * 64 : (ap + 1) * 64],
                        xsq[:, ap * BL : (ap + 1) * BL],
                        start=False, stop=True, tile_position=(0, 32 * je),
                    )
                    if jo % AG == 3:
                        g = ae // AG
                        v = work.tile([128, BL], bf16, tag="vmul")
                        nc.vector.tensor_mul(v[:], c4[:], y4[:])
                        o4 = psum_o.tile([AG, BL], f32)
                        nc.tensor.matmul(
                            o4[:], sig[:, :], v[:], start=True, stop=True
                        )
                        o4_sb = work.tile([AG, BL], f32, tag="osb")
                        nc.scalar.copy(o4_sb[:], o4[:])
                        nc.sync.dma_start(
                            out=out_d[:, g * BL : (g + 1) * BL], in_=o4_sb[:]
                        )

                return gathers, chain, contracts

            def unit_p3(ap, h):
                ae, ao = 2 * ap, 2 * ap + 1
                je, jo = ae % AG, ao % AG
                xa_e = xt[0:64, ap * BL : (ap + 1) * BL]
                xa_o = xt[64:128, ap * BL : (ap + 1) * BL]

                def gathers():
                    gb = psum_g.tile([128, 2 * BL], f32, tag="gath")
                    ga = psum_g.tile([128, 2 * BL], f32, tag="gath")
                    gc = psum_g.tile([128, 2 * BL], f32, tag="gath")
                    for po, xa, gs in (
                        (0, xa_e, slice(0, BL)),
                        (64, xa_o, slice(BL, 2 * BL)),
                    ):
                        rs = slice(po, po + 64)
                        nc.tensor.matmul(
                            gb[:, gs], sel3[rs, 256 + 128 * h : 384 + 128 * h],
                            xa, start=True, stop=True, tile_position=(po, 0),
                        )
                        nc.tensor.matmul(
                            ga[:, gs], sel3[rs, 128 * h : 128 * h + 128],
                            xa, start=True, stop=True, tile_position=(po, 0),
                        )
                        nc.tensor.matmul(
                            gc[:, gs], sel3[rs, 512 + 128 * h : 640 + 128 * h],
                            xa, start=True, stop=True, tile_position=(po, 0),
                        )
                    return (gb, ga, gc)

                def chain(g):
                    gb, ga, gc = g
                    gb_sb = work.tile([128, 2 * BL], bf16, tag="gsb")
                    nc.scalar.copy(gb_sb[:], gb[:])
                    if h == 0:
                        # both pair factors via SBUF so TT1 runs in the
                        # DVE 2x bf16 mode; copies ride the Scalar engine
                        ga_sb = work.tile([128, 2 * BL], bf16, tag="gasb")
                        nc.scalar.copy(ga_sb[:], ga[:])
                        t_sb = work.tile([128, 2 * BL], bf16, tag="xprod")
                        nc.vector.tensor_mul(t_sb[:], ga_sb[:], gb_sb[:])
                    else:
                        t_sb = work.tile([128, 2 * BL], bf16, tag="xprod")
                        nc.vector.tensor_mul(t_sb[:], ga[:], gb_sb[:])
                    x3 = work.tile([128, 2 * BL], bf16, tag="xprod")
                    nc.vector.tensor_mul(x3[:], gc[:], t_sb[:])
                    return x3

                def contracts(x3):
                    c4 = group_c4(ae // AG)
                    cs_e = c4[32 * je : 32 * je + 32, :]
                    cs_o = c4[32 * jo : 32 * jo + 32, :]
                    nc.tensor.matmul(
                        cs_e, wg3[:, ae * 64 + 32 * h : ae * 64 + 32 * h + 32],
                        x3[:, 0:BL],
                        start=(h == 0), stop=False, tile_position=(0, 32 * je),
                    )
                    nc.tensor.matmul(
                        cs_o, wg3[:, ao * 64 + 32 * h : ao * 64 + 32 * h + 32],
                        x3[:, BL : 2 * BL],
                        start=(h == 0), stop=False, tile_position=(0, 32 * jo),
                    )

                return gathers, chain, contracts

            units = []
            for ap in range(A // 2):
                units.append(unit_p3(ap, 0))
                units.append(unit_p3(ap, 1))
                units.append(unit_p2(ap))

            LAG = 2
            pending = []  # (chain_out, contracts) ring
            for gathers, chain, contracts in units:
                gout = gathers()
                cout = chain(gout)
                pending.append((cout, contracts))
                if len(pending) > LAG:
                    out0, k0 = pending.pop(0)
                    k0(out0)
            for out0, k0 in pending:
                k0(out0)

    nc.compile()
    return nc


def _host_prepare(x, y, w1, w2, w3, coeff1, coeff2, coeff3, idx):
    """Build per-core input maps (bf16 numpy via ml_dtypes)."""
    import ml_dtypes

    bf = ml_dtypes.bfloat16

    (i1, l1, k1, q1, i2, j2, l2, m2, k2, q2,
     i3, j3, f3, l3, m3, g3, k3, q3) = idx

    xf = np.ascontiguousarray(x.reshape(B, A, L * M), dtype=np.float32)
    c1 = i1 * M + l1
    c2a = i2 * M + l2
    c2b = j2 * M + m2
    c3a = i3 * M + l3
    c3b = j3 * M + m3
    c3c = f3 * M + g3

    # sel2: 2-hot (or single 2) columns for s = x[c2a] + x[c2b]
    sel2h = np.zeros((64, 128), dtype=np.float32)
    np.add.at(sel2h, (c2a, np.arange(P2)), 1.0)
    np.add.at(sel2h, (c2b, np.arange(P2)), 1.0)
    sel2 = np.concatenate([sel2h, sel2h], axis=0).astype(bf)

    # sel3 one-hot gathers: cols [h*128 + p] for factor a, +256 b, +512 c
    sel3h = np.zeros((64, 768), dtype=np.float32)
    pa = np.arange(P3)
    sel3h[c3a, pa] = 1.0
    sel3h[c3b, 256 + pa] = 1.0
    sel3h[c3c, 512 + pa] = 1.0
    sel3 = np.concatenate([sel3h, sel3h], axis=0).astype(bf)

    def blockdiag_pairs(m3):
        """[64, A, 32] folded stationary -> per-pair block-diagonal
        [128, (A//2)*64]: rows 0-63 even-a block in cols 0:32, rows
        64-127 odd-a block in cols 32:64 of each pair's 64-col slot."""
        out = np.zeros((128, (A // 2) * 64), dtype=np.float32)
        for ap in range(A // 2):
            out[0:64, ap * 64 : ap * 64 + 32] = m3[:, 2 * ap, :]
            out[64:128, ap * 64 + 32 : ap * 64 + 64] = m3[:, 2 * ap + 1, :]
        return out

    # a1w[c, a*32+e] = sum_{p: c1[p]=c} coeff1[p] * w1[e, k1[p], q1[p], a]
    W1g = (w1[:, k1, q1, :] * coeff1[None, :, None]).transpose(1, 2, 0)  # [P1,A,E]
    a1w3 = np.zeros((64, A, 32), dtype=np.float32)
    np.add.at(a1w3[:, :, :E], c1, W1g)
    a1w = blockdiag_pairs(a1w3).astype(bf)

    # path2: wg2 = W2g/2 (contract on s^2); sqw = -fold(W2g/2) (on xsq)
    W2g = (w2[:, k2, q2, :] * coeff2[None, :, None]).transpose(1, 2, 0)  # [P2,A,E]
    wg2_3 = np.zeros((P2, A, 32), dtype=np.float32)
    wg2_3[:, :, :E] = 0.5 * W2g
    wg2 = wg2_3.reshape(P2, A * 32).astype(bf)
    sqw3 = np.zeros((64, A, 32), dtype=np.float32)
    np.add.at(sqw3[:, :, :E], c2a, -0.5 * W2g)
    np.add.at(sqw3[:, :, :E], c2b, -0.5 * W2g)
    sqw = blockdiag_pairs(sqw3).astype(bf)

    # path3 contract weights: col (a*64 + 32h + e), rows = chunk paths
    W3g = (w3[:, k3, q3, :] * coeff3[None, :, None]).transpose(1, 2, 0)  # [P3,A,E]
    wg3_3 = np.zeros((128, A, 64), dtype=np.float32)
    wg3_3[:, :, 0:E] = W3g[:128]
    wg3_3[:, :, 32 : 32 + E] = W3g[128:]
    wg3 = wg3_3.reshape(128, A * 64).astype(bf)

    sig = np.zeros((128, 4), dtype=np.float32)
    for j in range(AG):
        sig[32 * j : 32 * j + E, j] = 1.0
    sig = sig.astype(bf)

    in_maps = []
    for k in range(NCORES):
        xb = xf[k * BL : (k + 1) * BL]  # [BL, A, 64]
        xtf = xb.transpose(2, 1, 0)  # [c, a, b]
        xt = np.empty((128, (A // 2) * BL), dtype=np.float32)
        xt[:64] = np.ascontiguousarray(xtf[:, 0::2, :]).reshape(64, (A // 2) * BL)
        xt[64:] = np.ascontiguousarray(xtf[:, 1::2, :]).reshape(64, (A // 2) * BL)
        yb = np.asarray(y[k * BL : (k + 1) * BL], dtype=np.float32)  # [BL, E]
        y4 = np.zeros((128, BL), dtype=np.float32)
        for j in range(AG):
            y4[32 * j : 32 * j + E, :] = yb.T
        in_maps.append(
            {
                "xt": xt.astype(bf), "y4": y4.astype(bf), "sel2": sel2,
                "sel3": sel3, "a1w": a1w, "sqw": sqw, "wg2": wg2,
                "wg3": wg3, "sig": sig,
            }
        )
    return in_maps


def _run(inputs: dict, trace: bool = False):
    from concourse.bass_utils import run_bass_kernel_spmd

    if "nc" not in _CACHE:
        _CACHE["nc"] = _build_module()
    nc = _CACHE["nc"]

    idx = tuple(
        np.asarray(inputs[k], dtype=np.int64)
        for k in ("i1", "l1", "k1", "q1", "i2", "j2", "l2", "m2", "k2", "q2",
                  "i3", "j3", "f3", "l3", "m3", "g3", "k3", "q3")
    )
    in_maps = _host_prepare(
        np.asarray(inputs["x"], np.float32),
        np.asarray(inputs["y"], np.float32),
        np.asarray(inputs["w1"], np.float32),
        np.asarray(inputs["w2"], np.float32),
        np.asarray(inputs["w3"], np.float32),
        np.asarray(inputs["coeff1"], np.float32),
        np.asarray(inputs["coeff2"], np.float32),
        np.asarray(inputs["coeff3"], np.float32),
        idx,
    )

    res = run_bass_kernel_spmd(nc, in_maps, core_ids=list(range(NCORES)), trace=trace)

    out = np.empty((B, A), dtype=np.float32)
    for k in range(NCORES):
        o = res.results[k]["out"]  # [4, NG*BL]
        o = o.reshape(AG, NG, BL)  # [j, g, b]
        t_core = o.transpose(1, 0, 2).reshape(A, BL)  # [a, b]
        out[k * BL : (k + 1) * BL, :] = t_core.T
    return out, res


def kernel(**inputs) -> np.ndarray:
    out, _ = _run(inputs, trace=False)
    return out
